# revision 13
# baseline (speedup 1.0000x reference)
import os
import sys

for _p in ("/opt/trn_rl_repo", "/root/.axon_site/_ro/trn_rl_repo"):
    if os.path.isdir(_p) and _p not in sys.path:
        sys.path.insert(0, _p)

import numpy as np
import concourse.bacc as bacc
import concourse.mybir as mybir
import concourse.tile as tile
from concourse import bass_utils

B, N, T, F = 8, 128, 2048, 32
L, H = 5, 64

FP32 = mybir.dt.float32
FP32R = mybir.dt.float32r

TT = 256          # t-steps per x tile
HALO = 4          # max_lag - 1
CHUNK = 16        # t-steps per output chunk
NTILES = T // TT  # 8
NCHUNKS = TT // CHUNK  # 16 per tile

X_TILE_FREE = (TT + HALO) * F  # 8320 floats per partition
Y_CHUNK_FREE = CHUNK * H       # 1024

_CACHE = {}
LAST_RESULTS = None


def _build_nc():
    nc = bacc.Bacc("TRN2", target_bir_lowering=False, debug=False)
    x_d = nc.dram_tensor("x", (N, T * F), FP32, kind="ExternalInput").ap()
    at_d = nc.dram_tensor("at", (N, L * N), FP32, kind="ExternalInput").ap()
    wblk_d = nc.dram_tensor("wblk", (128, 256), FP32, kind="ExternalInput").ap()
    brow_d = nc.dram_tensor("brow", (1, 256), FP32, kind="ExternalInput").ap()
    ones_d = nc.dram_tensor("ones", (1, 128), FP32, kind="ExternalInput").ap()
    zeros_d = nc.dram_tensor("zeros", (N, HALO * F), FP32, kind="ExternalInput").ap()
    ident_d = nc.dram_tensor("ident", (128, 128), FP32, kind="ExternalInput").ap()
    y_d = nc.dram_tensor("y", (N, T * H), FP32, kind="ExternalOutput").ap()

    gelu = mybir.ActivationFunctionType.Gelu

    with tile.TileContext(nc) as tc:
        with (
            tc.tile_pool(name="consts", bufs=1) as consts,
            tc.tile_pool(name="xpool", bufs=2) as xpool,
            tc.tile_pool(name="aggpool", bufs=3) as aggpool,
            tc.tile_pool(name="aggtpool", bufs=3) as aggtpool,
            tc.tile_pool(name="ypool", bufs=12) as ypool,
            tc.tile_pool(name="pagg", bufs=2, space="PSUM") as pagg,
            tc.tile_pool(name="pt", bufs=2, space="PSUM") as pt,
            tc.tile_pool(name="py", bufs=2, space="PSUM") as py,
        ):
            at_sb = consts.tile((N, L * N), FP32R)
            w_sb = consts.tile((128, 256), FP32R)
            brow_sb = consts.tile((1, 256), FP32)
            ones_sb = consts.tile((1, 128), FP32)
            ident_sb = consts.tile((128, 128), FP32R)
            nc.sync.dma_start(out=at_sb, in_=at_d.bitcast(FP32R))
            nc.sync.dma_start(out=w_sb, in_=wblk_d.bitcast(FP32R))
            nc.sync.dma_start(out=brow_sb, in_=brow_d)
            nc.sync.dma_start(out=ones_sb, in_=ones_d)
            nc.sync.dma_start(out=ident_sb, in_=ident_d.bitcast(FP32R))

            for ti in range(NTILES):
                t0 = ti * TT
                x_tile = xpool.tile((N, X_TILE_FREE), FP32R)
                if ti == 0:
                    nc.sync.dma_start(
                        out=x_tile[:, 0 : HALO * F], in_=zeros_d.bitcast(FP32R)
                    )
                    src0 = x_d[:, 0 : TT * F].bitcast(FP32R)
                    nslice = 8
                    sl = TT * F // nslice
                    for s in range(nslice):
                        nc.sync.dma_start(
                            out=x_tile[:, HALO * F + s * sl : HALO * F + (s + 1) * sl],
                            in_=src0[:, s * sl : (s + 1) * sl],
                        )
                else:
                    src = x_d[:, (t0 - HALO) * F : (t0 + TT) * F].bitcast(FP32R)
                    nslice = 8
                    sl = X_TILE_FREE // nslice
                    for s in range(nslice):
                        nc.sync.dma_start(
                            out=x_tile[:, s * sl : (s + 1) * sl],
                            in_=src[:, s * sl : (s + 1) * sl],
                        )

                for c in range(NCHUNKS):
                    psum_agg = pagg.tile((N, 512), FP32)
                    for lag in range(L):
                        off = (HALO + CHUNK * c - lag) * F
                        nc.tensor.matmul(
                            psum_agg,
                            at_sb[:, lag * N : (lag + 1) * N],
                            x_tile[:, off : off + 512],
                            start=(lag == 0),
                            stop=(lag == L - 1),
                        )
                    sbuf_agg = aggpool.tile((N, 512), FP32R)
                    nc.scalar.copy(sbuf_agg, psum_agg)

                    sbuf_aggt = aggtpool.tile((N, 512), FP32R)
                    psum_y = py.tile((N, Y_CHUNK_FREE), FP32)
                    for s in range(4):
                        psum_t = pt.tile((128, 128), FP32R)
                        nc.tensor.transpose(
                            psum_t, sbuf_agg[:, s * 128 : (s + 1) * 128], ident_sb
                        )
                        nc.vector.tensor_copy(
                            sbuf_aggt[:, s * 128 : (s + 1) * 128], psum_t
                        )
                        nc.tensor.matmul(
                            psum_y[:, s * 256 : (s + 1) * 256],
                            ones_sb,
                            brow_sb,
                            start=True,
                            stop=False,
                        )
                        nc.tensor.matmul(
                            psum_y[:, s * 256 : (s + 1) * 256],
                            sbuf_aggt[:, s * 128 : (s + 1) * 128],
                            w_sb,
                            start=False,
                            stop=True,
                        )

                    sbuf_y = ypool.tile((N, Y_CHUNK_FREE), FP32)
                    nc.scalar.activation(sbuf_y, psum_y, func=gelu)
                    nc.sync.dma_start(
                        out=y_d[:, (t0 + c * CHUNK) * H : (t0 + (c + 1) * CHUNK) * H],
                        in_=sbuf_y,
                    )
    nc.compile()
    return nc


def kernel(x, A_list, W, b):
    global LAST_RESULTS
    x = np.asarray(x, np.float32)
    A_list = np.asarray(A_list, np.float32)
    W = np.asarray(W, np.float32)
    b = np.asarray(b, np.float32)

    if "nc" not in _CACHE:
        _CACHE["nc"] = _build_nc()
    nc = _CACHE["nc"]

    wblk = np.zeros((128, 256), np.float32)
    for q in range(4):
        wblk[q * F : (q + 1) * F, q * H : (q + 1) * H] = W.T
    brow = np.ascontiguousarray(np.tile(b, 4)[None, :])
    ones = np.ones((1, 128), np.float32)
    zeros = np.zeros((N, HALO * F), np.float32)
    ident = np.eye(128, dtype=np.float32)

    in_maps = []
    for c in range(B):
        in_maps.append(
            {
                "x": x[c].reshape(N, T * F),
                "at": np.ascontiguousarray(
                    A_list[c].transpose(2, 0, 1).reshape(N, L * N)
                ),
                "wblk": wblk,
                "brow": brow,
                "ones": ones,
                "zeros": zeros,
                "ident": ident,
            }
        )

    trace = bool(os.environ.get("KERNEL_TRACE"))
    res = bass_utils.run_bass_kernel_spmd(
        nc, in_maps, core_ids=list(range(B)), trace=trace
    )
    LAST_RESULTS = res
    out = np.stack([res.results[c]["y"].reshape(N, T, H) for c in range(B)])
    return out


# revision 19
# speedup vs baseline: 3.4387x; 3.4387x over previous
import os
import sys

for _p in ("/opt/trn_rl_repo", "/root/.axon_site/_ro/trn_rl_repo"):
    if os.path.isdir(_p) and _p not in sys.path:
        sys.path.insert(0, _p)

import numpy as np
import concourse.bacc as bacc
import concourse.mybir as mybir
import concourse.tile as tile
from concourse import bass_utils

B, N, T, F = 8, 128, 2048, 32
L, H = 5, 64

FP32 = mybir.dt.float32
FP32R = mybir.dt.float32r

TT = 256          # t-steps per x tile
HALO = 4          # max_lag - 1
CHUNK = 16        # t-steps per output chunk
NTILES = T // TT  # 8
NCHUNKS = TT // CHUNK  # 16 per tile

X_TILE_FREE = (TT + HALO) * F  # 8320 floats per partition
Y_CHUNK_FREE = CHUNK * H       # 1024

_CACHE = {}
LAST_RESULTS = None


def _build_nc():
    nc = bacc.Bacc("TRN2", target_bir_lowering=False, debug=False)
    x_d = nc.dram_tensor("x", (N, T * F), FP32, kind="ExternalInput").ap()
    at_d = nc.dram_tensor("at", (N, L * N), FP32, kind="ExternalInput").ap()
    wblk_d = nc.dram_tensor("wblk", (128, 256), FP32, kind="ExternalInput").ap()
    btile_d = nc.dram_tensor("btile", (N, Y_CHUNK_FREE), FP32, kind="ExternalInput").ap()
    zeros_d = nc.dram_tensor("zeros", (N, HALO * F), FP32, kind="ExternalInput").ap()
    ident_d = nc.dram_tensor("ident", (128, 128), FP32, kind="ExternalInput").ap()
    y_d = nc.dram_tensor("y", (N, T * H), FP32, kind="ExternalOutput").ap()

    gelu = mybir.ActivationFunctionType.Gelu
    add = mybir.AluOpType.add

    with tile.TileContext(nc) as tc:
        with (
            tc.tile_pool(name="consts", bufs=1) as consts,
            tc.tile_pool(name="xpool", bufs=2) as xpool,
            tc.tile_pool(name="aggpool", bufs=3) as aggpool,
            tc.tile_pool(name="aggtpool", bufs=3) as aggtpool,
            tc.tile_pool(name="ypool", bufs=12) as ypool,
            tc.tile_pool(name="pagg", bufs=2, space="PSUM") as pagg,
            tc.tile_pool(name="pt", bufs=2, space="PSUM") as pt,
            tc.tile_pool(name="py", bufs=2, space="PSUM") as py,
        ):
            at_sb = consts.tile((N, L * N), FP32R)
            w_sb = consts.tile((128, 256), FP32R)
            btile_sb = consts.tile((N, Y_CHUNK_FREE), FP32)
            ident_sb = consts.tile((128, 128), FP32R)
            nc.sync.dma_start(out=at_sb, in_=at_d.bitcast(FP32R))
            nc.sync.dma_start(out=w_sb, in_=wblk_d.bitcast(FP32R))
            nc.sync.dma_start(out=btile_sb, in_=btile_d)
            nc.sync.dma_start(out=ident_sb, in_=ident_d.bitcast(FP32R))

            for ti in range(NTILES):
                t0 = ti * TT
                x_tile = xpool.tile((N, X_TILE_FREE), FP32R)
                if ti == 0:
                    nc.sync.dma_start(
                        out=x_tile[:, 0 : HALO * F], in_=zeros_d.bitcast(FP32R)
                    )
                    src0 = x_d[:, 0 : TT * F].bitcast(FP32R)
                    nslice = 8
                    sl = TT * F // nslice
                    for s in range(nslice):
                        nc.sync.dma_start(
                            out=x_tile[:, HALO * F + s * sl : HALO * F + (s + 1) * sl],
                            in_=src0[:, s * sl : (s + 1) * sl],
                        )
                else:
                    src = x_d[:, (t0 - HALO) * F : (t0 + TT) * F].bitcast(FP32R)
                    nslice = 8
                    sl = X_TILE_FREE // nslice
                    for s in range(nslice):
                        nc.sync.dma_start(
                            out=x_tile[:, s * sl : (s + 1) * sl],
                            in_=src[:, s * sl : (s + 1) * sl],
                        )

                for c in range(NCHUNKS):
                    psum_agg = pagg.tile((N, 512), FP32)
                    for lag in range(L):
                        off = (HALO + CHUNK * c - lag) * F
                        nc.tensor.matmul(
                            psum_agg,
                            at_sb[:, lag * N : (lag + 1) * N],
                            x_tile[:, off : off + 512],
                            start=(lag == 0),
                            stop=(lag == L - 1),
                        )
                    sbuf_agg = aggpool.tile((N, 512), FP32R)
                    nc.scalar.copy(sbuf_agg, psum_agg)

                    psum_t = pt.tile((128, 512), FP32R)
                    for s in range(4):
                        nc.tensor.transpose(
                            psum_t[:, s * 128 : (s + 1) * 128],
                            sbuf_agg[:, s * 128 : (s + 1) * 128],
                            ident_sb,
                        )
                    sbuf_aggt = aggtpool.tile((N, 512), FP32R)
                    nc.vector.tensor_copy(sbuf_aggt, psum_t)

                    psum_y = py.tile((N, Y_CHUNK_FREE), FP32)
                    for s in range(4):
                        nc.tensor.matmul(
                            psum_y[:, s * 256 : (s + 1) * 256],
                            sbuf_aggt[:, s * 128 : (s + 1) * 128],
                            w_sb,
                            start=True,
                            stop=True,
                        )

                    sbuf_y = ypool.tile((N, Y_CHUNK_FREE), FP32)
                    nc.vector.tensor_tensor(sbuf_y, psum_y, btile_sb, add)
                    nc.scalar.activation(sbuf_y, sbuf_y, func=gelu)
                    nc.sync.dma_start(
                        out=y_d[:, (t0 + c * CHUNK) * H : (t0 + (c + 1) * CHUNK) * H],
                        in_=sbuf_y,
                    )
    nc.compile()
    return nc


def kernel(x, A_list, W, b):
    global LAST_RESULTS
    x = np.asarray(x, np.float32)
    A_list = np.asarray(A_list, np.float32)
    W = np.asarray(W, np.float32)
    b = np.asarray(b, np.float32)

    if "nc" not in _CACHE:
        _CACHE["nc"] = _build_nc()
    nc = _CACHE["nc"]

    wblk = np.zeros((128, 256), np.float32)
    for q in range(4):
        wblk[q * F : (q + 1) * F, q * H : (q + 1) * H] = W.T
    btile = np.ascontiguousarray(
        np.broadcast_to(np.tile(b, CHUNK)[None, :], (N, Y_CHUNK_FREE))
    )
    zeros = np.zeros((N, HALO * F), np.float32)
    ident = np.eye(128, dtype=np.float32)

    in_maps = []
    for c in range(B):
        in_maps.append(
            {
                "x": x[c].reshape(N, T * F),
                "at": np.ascontiguousarray(
                    A_list[c].transpose(2, 0, 1).reshape(N, L * N)
                ),
                "wblk": wblk,
                "btile": btile,
                "zeros": zeros,
                "ident": ident,
            }
        )

    trace = bool(os.environ.get("KERNEL_TRACE"))
    res = bass_utils.run_bass_kernel_spmd(
        nc, in_maps, core_ids=list(range(B)), trace=trace
    )
    LAST_RESULTS = res
    out = np.stack([res.results[c]["y"].reshape(N, T, H) for c in range(B)])
    return out


# revision 20
# speedup vs baseline: 3.8486x; 1.1192x over previous
import os
import sys

for _p in ("/opt/trn_rl_repo", "/root/.axon_site/_ro/trn_rl_repo"):
    if os.path.isdir(_p) and _p not in sys.path:
        sys.path.insert(0, _p)

import numpy as np
import concourse.bacc as bacc
import concourse.mybir as mybir
import concourse.tile as tile
from concourse import bass_utils

B, N, T, F = 8, 128, 2048, 32
L, H = 5, 64

FP32 = mybir.dt.float32
FP32R = mybir.dt.float32r

TT = 256          # t-steps per x tile
HALO = 4          # max_lag - 1
CHUNK = 16        # t-steps per output chunk
NTILES = T // TT  # 8
NCHUNKS = TT // CHUNK  # 16 per tile

X_TILE_FREE = (TT + HALO) * F  # 8320 floats per partition
Y_CHUNK_FREE = CHUNK * H       # 1024

_CACHE = {}
LAST_RESULTS = None


def _build_nc():
    nc = bacc.Bacc("TRN2", target_bir_lowering=False, debug=False)
    x_d = nc.dram_tensor("x", (N, T * F), FP32, kind="ExternalInput").ap()
    at_d = nc.dram_tensor("at", (N, L * N), FP32, kind="ExternalInput").ap()
    wblk_d = nc.dram_tensor("wblk", (128, 256), FP32, kind="ExternalInput").ap()
    btile_d = nc.dram_tensor("btile", (N, Y_CHUNK_FREE), FP32, kind="ExternalInput").ap()
    zeros_d = nc.dram_tensor("zeros", (N, HALO * F), FP32, kind="ExternalInput").ap()
    ident_d = nc.dram_tensor("ident", (128, 128), FP32, kind="ExternalInput").ap()
    y_d = nc.dram_tensor("y", (N, T * H), FP32, kind="ExternalOutput").ap()

    gelu = mybir.ActivationFunctionType.Gelu
    add = mybir.AluOpType.add

    with tile.TileContext(nc) as tc:
        with (
            tc.tile_pool(name="consts", bufs=1) as consts,
            tc.tile_pool(name="xpool", bufs=2) as xpool,
            tc.tile_pool(name="aggpool", bufs=3) as aggpool,
            tc.tile_pool(name="aggtpool", bufs=3) as aggtpool,
            tc.tile_pool(name="ypool", bufs=12) as ypool,
            tc.tile_pool(name="pagg", bufs=2, space="PSUM") as pagg,
            tc.tile_pool(name="pt", bufs=2, space="PSUM") as pt,
            tc.tile_pool(name="py", bufs=2, space="PSUM") as py,
        ):
            at_sb = consts.tile((N, L * N), FP32R)
            w_sb = consts.tile((128, 256), FP32R)
            btile_sb = consts.tile((N, Y_CHUNK_FREE), FP32)
            ident_sb = consts.tile((128, 128), FP32R)
            nc.sync.dma_start(out=at_sb, in_=at_d.bitcast(FP32R))
            nc.sync.dma_start(out=w_sb, in_=wblk_d.bitcast(FP32R))
            nc.sync.dma_start(out=btile_sb, in_=btile_d)
            nc.sync.dma_start(out=ident_sb, in_=ident_d.bitcast(FP32R))

            x_tiles = {}
            agg_of = {}
            aggt_of = {}
            TOTAL = NTILES * NCHUNKS

            def emit_xload(ti):
                x_tile = xpool.tile((N, X_TILE_FREE), FP32R)
                x_tiles[ti] = x_tile
                t0 = ti * TT
                if ti == 0:
                    nc.sync.dma_start(
                        out=x_tile[:, 0 : HALO * F], in_=zeros_d.bitcast(FP32R)
                    )
                    src0 = x_d[:, 0 : TT * F].bitcast(FP32R)
                    sl = TT * F // 8
                    for s in range(8):
                        nc.sync.dma_start(
                            out=x_tile[:, HALO * F + s * sl : HALO * F + (s + 1) * sl],
                            in_=src0[:, s * sl : (s + 1) * sl],
                        )
                else:
                    src = x_d[:, (t0 - HALO) * F : (t0 + TT) * F].bitcast(FP32R)
                    sl = X_TILE_FREE // 8
                    for s in range(8):
                        nc.sync.dma_start(
                            out=x_tile[:, s * sl : (s + 1) * sl],
                            in_=src[:, s * sl : (s + 1) * sl],
                        )

            def emit_s1(g):
                ti, c = divmod(g, NCHUNKS)
                x_tile = x_tiles[ti]
                psum_agg = pagg.tile((N, 512), FP32)
                for lag in range(L):
                    off = (HALO + CHUNK * c - lag) * F
                    nc.tensor.matmul(
                        psum_agg,
                        at_sb[:, lag * N : (lag + 1) * N],
                        x_tile[:, off : off + 512],
                        start=(lag == 0),
                        stop=(lag == L - 1),
                    )
                sbuf_agg = aggpool.tile((N, 512), FP32R)
                nc.scalar.copy(sbuf_agg, psum_agg)
                agg_of[g] = sbuf_agg

            def emit_tr(g):
                sbuf_agg = agg_of.pop(g)
                psum_t = pt.tile((128, 512), FP32R)
                for s in range(4):
                    nc.tensor.transpose(
                        psum_t[:, s * 128 : (s + 1) * 128],
                        sbuf_agg[:, s * 128 : (s + 1) * 128],
                        ident_sb,
                    )
                sbuf_aggt = aggtpool.tile((N, 512), FP32R)
                nc.vector.tensor_copy(sbuf_aggt, psum_t)
                aggt_of[g] = sbuf_aggt

            def emit_s2(g):
                sbuf_aggt = aggt_of.pop(g)
                psum_y = py.tile((N, Y_CHUNK_FREE), FP32)
                for s in range(4):
                    nc.tensor.matmul(
                        psum_y[:, s * 256 : (s + 1) * 256],
                        sbuf_aggt[:, s * 128 : (s + 1) * 128],
                        w_sb,
                        start=True,
                        stop=True,
                    )
                sbuf_y = ypool.tile((N, Y_CHUNK_FREE), FP32)
                nc.vector.tensor_tensor(sbuf_y, psum_y, btile_sb, add)
                nc.scalar.activation(sbuf_y, sbuf_y, func=gelu)
                nc.sync.dma_start(
                    out=y_d[:, g * CHUNK * H : (g + 1) * CHUNK * H], in_=sbuf_y
                )

            emit_xload(0)
            for g in range(TOTAL + 2):
                if g < TOTAL:
                    ti, c = divmod(g, NCHUNKS)
                    emit_s1(g)
                    if c == 0 and ti + 1 < NTILES:
                        emit_xload(ti + 1)
                if 1 <= g <= TOTAL:
                    emit_tr(g - 1)
                if g >= 2:
                    emit_s2(g - 2)
    nc.compile()
    return nc


def kernel(x, A_list, W, b):
    global LAST_RESULTS
    x = np.asarray(x, np.float32)
    A_list = np.asarray(A_list, np.float32)
    W = np.asarray(W, np.float32)
    b = np.asarray(b, np.float32)

    if "nc" not in _CACHE:
        _CACHE["nc"] = _build_nc()
    nc = _CACHE["nc"]

    wblk = np.zeros((128, 256), np.float32)
    for q in range(4):
        wblk[q * F : (q + 1) * F, q * H : (q + 1) * H] = W.T
    btile = np.ascontiguousarray(
        np.broadcast_to(np.tile(b, CHUNK)[None, :], (N, Y_CHUNK_FREE))
    )
    zeros = np.zeros((N, HALO * F), np.float32)
    ident = np.eye(128, dtype=np.float32)

    in_maps = []
    for c in range(B):
        in_maps.append(
            {
                "x": x[c].reshape(N, T * F),
                "at": np.ascontiguousarray(
                    A_list[c].transpose(2, 0, 1).reshape(N, L * N)
                ),
                "wblk": wblk,
                "btile": btile,
                "zeros": zeros,
                "ident": ident,
            }
        )

    trace = bool(os.environ.get("KERNEL_TRACE"))
    res = bass_utils.run_bass_kernel_spmd(
        nc, in_maps, core_ids=list(range(B)), trace=trace
    )
    LAST_RESULTS = res
    out = np.stack([res.results[c]["y"].reshape(N, T, H) for c in range(B)])
    return out


# revision 24
# speedup vs baseline: 4.6948x; 1.2199x over previous
import os
import sys

for _p in ("/opt/trn_rl_repo", "/root/.axon_site/_ro/trn_rl_repo"):
    if os.path.isdir(_p) and _p not in sys.path:
        sys.path.insert(0, _p)

import numpy as np
import concourse.bacc as bacc
import concourse.mybir as mybir
import concourse.tile as tile
from concourse import bass_utils

B, N, T, F = 8, 128, 2048, 32
L, H = 5, 64

FP32 = mybir.dt.float32
FP32R = mybir.dt.float32r
FP16 = mybir.dt.float16

TT = 256          # t-steps per x tile
HALO = 4          # max_lag - 1
CHUNK = 16        # t-steps per output chunk
NTILES = T // TT  # 8
NCHUNKS = TT // CHUNK  # 16 per tile

X_TILE_FREE = (TT + HALO) * F  # 8320 floats per partition
Y_CHUNK_FREE = CHUNK * H       # 1024

_CACHE = {}
LAST_RESULTS = None


def _build_nc():
    nc = bacc.Bacc("TRN2", target_bir_lowering=False, debug=False)
    x_d = nc.dram_tensor("x", (N, T * F), FP32, kind="ExternalInput").ap()
    at_d = nc.dram_tensor("at", (N, L * N), FP32, kind="ExternalInput").ap()
    wblk_d = nc.dram_tensor("wblk", (128, 256), FP32, kind="ExternalInput").ap()
    btile_d = nc.dram_tensor("btile", (N, Y_CHUNK_FREE), FP32, kind="ExternalInput").ap()
    zeros_d = nc.dram_tensor("zeros", (N, HALO * F), FP32, kind="ExternalInput").ap()
    ident_d = nc.dram_tensor("ident", (128, 128), FP32, kind="ExternalInput").ap()
    y_d = nc.dram_tensor("y", (N, T * H), FP16, kind="ExternalOutput").ap()

    gelu = mybir.ActivationFunctionType.Gelu
    add = mybir.AluOpType.add

    with tile.TileContext(nc) as tc:
        with (
            tc.tile_pool(name="consts", bufs=1) as consts,
            tc.tile_pool(name="xpool", bufs=2) as xpool,
            tc.tile_pool(name="aggpool", bufs=3) as aggpool,
            tc.tile_pool(name="aggtpool", bufs=3) as aggtpool,
            tc.tile_pool(name="ypool", bufs=12) as ypool,
            tc.tile_pool(name="pagg", bufs=2, space="PSUM") as pagg,
            tc.tile_pool(name="pt", bufs=2, space="PSUM") as pt,
            tc.tile_pool(name="py", bufs=2, space="PSUM") as py,
        ):
            at_sb = consts.tile((N, L * N), FP32R)
            w_sb = consts.tile((128, 256), FP32R)
            btile_sb = consts.tile((N, Y_CHUNK_FREE), FP32)
            ident_sb = consts.tile((128, 128), FP32R)
            nc.sync.dma_start(out=at_sb, in_=at_d.bitcast(FP32R))
            nc.sync.dma_start(out=w_sb, in_=wblk_d.bitcast(FP32R))
            nc.sync.dma_start(out=btile_sb, in_=btile_d)
            nc.sync.dma_start(out=ident_sb, in_=ident_d.bitcast(FP32R))

            x_tiles = {}
            agg_of = {}
            aggt_of = {}
            TOTAL = NTILES * NCHUNKS

            def emit_xload(ti):
                x_tile = xpool.tile((N, X_TILE_FREE), FP32R)
                x_tiles[ti] = x_tile
                t0 = ti * TT
                if ti == 0:
                    nc.sync.dma_start(
                        out=x_tile[:, 0 : HALO * F], in_=zeros_d.bitcast(FP32R)
                    )
                    src0 = x_d[:, 0 : TT * F].bitcast(FP32R)
                    sl = TT * F // 8
                    for s in range(8):
                        nc.sync.dma_start(
                            out=x_tile[:, HALO * F + s * sl : HALO * F + (s + 1) * sl],
                            in_=src0[:, s * sl : (s + 1) * sl],
                        )
                else:
                    src = x_d[:, (t0 - HALO) * F : (t0 + TT) * F].bitcast(FP32R)
                    sl = X_TILE_FREE // 8
                    for s in range(8):
                        nc.sync.dma_start(
                            out=x_tile[:, s * sl : (s + 1) * sl],
                            in_=src[:, s * sl : (s + 1) * sl],
                        )

            def emit_s1(g):
                ti, c = divmod(g, NCHUNKS)
                x_tile = x_tiles[ti]
                psum_agg = pagg.tile((N, 512), FP32)
                for lag in range(L):
                    off = (HALO + CHUNK * c - lag) * F
                    nc.tensor.matmul(
                        psum_agg,
                        at_sb[:, lag * N : (lag + 1) * N],
                        x_tile[:, off : off + 512],
                        start=(lag == 0),
                        stop=(lag == L - 1),
                    )
                sbuf_agg = aggpool.tile((N, 512), FP32R)
                nc.scalar.copy(sbuf_agg, psum_agg)
                agg_of[g] = sbuf_agg

            def emit_tr(g):
                sbuf_agg = agg_of.pop(g)
                psum_t = pt.tile((128, 512), FP32R)
                for s in range(4):
                    nc.tensor.transpose(
                        psum_t[:, s * 128 : (s + 1) * 128],
                        sbuf_agg[:, s * 128 : (s + 1) * 128],
                        ident_sb,
                    )
                sbuf_aggt = aggtpool.tile((N, 512), FP32R)
                nc.vector.tensor_copy(sbuf_aggt, psum_t)
                aggt_of[g] = sbuf_aggt

            def emit_s2(g):
                sbuf_aggt = aggt_of.pop(g)
                psum_y = py.tile((N, Y_CHUNK_FREE), FP32)
                for s in range(4):
                    nc.tensor.matmul(
                        psum_y[:, s * 256 : (s + 1) * 256],
                        sbuf_aggt[:, s * 128 : (s + 1) * 128],
                        w_sb,
                        start=True,
                        stop=True,
                    )
                sbuf_y = ypool.tile((N, Y_CHUNK_FREE), FP16)
                nc.vector.tensor_tensor(sbuf_y, psum_y, btile_sb, add)
                nc.scalar.activation(sbuf_y, sbuf_y, func=gelu)
                nc.sync.dma_start(
                    out=y_d[:, g * CHUNK * H : (g + 1) * CHUNK * H], in_=sbuf_y
                )

            emit_xload(0)
            for g in range(TOTAL + 2):
                if g < TOTAL:
                    ti, c = divmod(g, NCHUNKS)
                    emit_s1(g)
                    if c == 0 and ti + 1 < NTILES:
                        emit_xload(ti + 1)
                if 1 <= g <= TOTAL:
                    emit_tr(g - 1)
                if g >= 2:
                    emit_s2(g - 2)
    nc.compile()
    return nc


def kernel(x, A_list, W, b):
    global LAST_RESULTS
    x = np.asarray(x, np.float32)
    A_list = np.asarray(A_list, np.float32)
    W = np.asarray(W, np.float32)
    b = np.asarray(b, np.float32)

    if "nc" not in _CACHE:
        _CACHE["nc"] = _build_nc()
    nc = _CACHE["nc"]

    wblk = np.zeros((128, 256), np.float32)
    for q in range(4):
        wblk[q * F : (q + 1) * F, q * H : (q + 1) * H] = W.T
    btile = np.ascontiguousarray(
        np.broadcast_to(np.tile(b, CHUNK)[None, :], (N, Y_CHUNK_FREE))
    )
    zeros = np.zeros((N, HALO * F), np.float32)
    ident = np.eye(128, dtype=np.float32)

    in_maps = []
    for c in range(B):
        in_maps.append(
            {
                "x": x[c].reshape(N, T * F),
                "at": np.ascontiguousarray(
                    A_list[c].transpose(2, 0, 1).reshape(N, L * N)
                ),
                "wblk": wblk,
                "btile": btile,
                "zeros": zeros,
                "ident": ident,
            }
        )

    trace = bool(os.environ.get("KERNEL_TRACE"))
    res = bass_utils.run_bass_kernel_spmd(
        nc, in_maps, core_ids=list(range(B)), trace=trace
    )
    LAST_RESULTS = res
    out = np.stack(
        [
            np.asarray(res.results[c]["y"], np.float32).reshape(N, T, H)
            for c in range(B)
        ]
    )
    return out
